# revision 51
# baseline (speedup 1.0000x reference)
"""Trainium2 Bass kernel for nn_AttentionLayer (pre-conv + self-attention + final conv).

Sharding: 8 cores = 2 samples x 4 query-row chunks. Each core computes the
full pre-conv y for its sample (k/v need all N=9216 positions), attention for
its 26-row query window (24 own rows + 1 halo row each side for the final
3x3 conv), and the final conv for its 24 output rows.

V1 perf structure (vs the 405us baseline):
- The PE engine clock ramps with sustained use (0.65 -> 1.2 -> 2.4 GHz after
  ~3us of continuous execution). The whole schedule is built to keep the PE
  busy back-to-back: conv/projection chunks interleave, and in the attention
  loop energy round g+1 is emitted before PV of round g.
- Energy matmuls run 4-way concurrent via tile_position row groups (K=16 at
  partition offsets 0/32/64/96), each writing its own 1-bank PSUM tile.
- exp splits across the two PSUM-capable engines: ACT does true Exp, DVE does
  a Schraudolph bit-trick exp (i16 = round(128*log2e*x + 16256 - 5.5) bitcast
  to bf16; ~3% max elem error, diluted ~90x by the diffuse softmax).
- Softmax denominator comes from an augmented ones-column in wv (scaled by
  1/gamma so the reciprocal directly yields gamma/s); the denominator
  broadcast is a K=1 ones matmul on the PE instead of a DRAM round trip.
- Final 3x3 conv is 2-row-packed (6 matmuls instead of 9) from a
  double-height padded buffer filled by the Pool engine.
"""

import os
import hashlib
import shutil

import numpy as np
import ml_dtypes

BF16 = ml_dtypes.bfloat16
EPS = 1e-5

B, C, CQK, H, W = 2, 64, 16, 96, 96
N = H * W                       # 9216
QCH = 4                         # query chunks per sample
ROWS = H // QCH                 # 24 rows per core
LOCROWS = ROWS + 2              # 26 (with halo)
NLOC = LOCROWS * W              # 2496
HP, WP = H + 2, W + 2           # 98x98 padded frame
LOCP = LOCROWS + 2              # 28 padded local rows
NI_SIZES = [512, 512, 512, 512, 448]   # i-chunks over NLOC
JB = 128                        # j-block height
NJB = N // JB                   # 72
NPAIR = NJB // 2                # 36 DoubleRow j-block pairs
NRND = 18                       # rounds per i-chunk (2 pairs = 4 jb each)
VB = C + 1                      # 65
VBP = 80                        # padded vT slot width (DoubleRow step%16==0)

L2E = 1.4426950408889634
A16 = 128.0 * L2E
B16 = 127.0 * 128.0 - 5.5       # calibrated Schraudolph offset (bf16 variant)
A8 = 8.0 * L2E
B8 = 7.0 * 8.0 - 0.5            # calibrated fp8e4 Schraudolph offset

# final-conv chunks that become ready after attention i-chunk ci
FC_SCHED = {1: [0, 1], 2: [2], 3: [3], 4: [4, 5]}


# ---------------------------------------------------------------------------
# framework patches (self-contained)
# ---------------------------------------------------------------------------

def _apply_patches():
    import concourse.tile as tile
    import concourse.bass_utils as bu
    import concourse.bass2jax as b2j
    from concourse import mybir

    # 1) walrus in this env rejects >1-2 sync waits on the final Drain
    #    (CTRL_NO_STRUCT): split waits into single-wait nops.
    def _drain_and_barrier_split(self, tick_clock, wait_clock):
        nc = self.nc
        probe = nc.sync.nop()
        wait_clock.add_sem_waits(
            probe.ins, tile.ScopedClock({None: tick_clock.global_clock})
        )
        waits = list(probe.ins.sync_info.on_wait) if probe.ins.sync_info else []
        if probe.ins.sync_info is not None:
            probe.ins.sync_info.on_wait = []
        for w in waits[:-1]:
            nop = nc.sync.nop()
            if nop.ins.sync_info is None:
                nop.ins.sync_info = mybir.SyncInfo(on_wait=[w], on_update=[])
            else:
                nop.ins.sync_info.on_wait.append(w)
        drain_inst = nc.sync.drain()
        if waits:
            if drain_inst.ins.sync_info is None:
                drain_inst.ins.sync_info = mybir.SyncInfo(
                    on_wait=[waits[-1]], on_update=[]
                )
            else:
                drain_inst.ins.sync_info.on_wait.append(waits[-1])
        nc.all_engine_barrier()
        assert self.sems is not None
        popped = nc._tile_sem_poison_stack.pop()
        assert popped is self._sem_poison
        nc.clear_and_free_semaphores(list(self.sems.allocated().values()))
        nc.all_engine_barrier()

    tile.TileContext._drain_and_barrier = _drain_and_barrier_split

    # 2) NEFF disk cache keyed by BIR hash (compile is deterministic).
    cache_dir = os.path.join(os.path.dirname(os.path.abspath(__file__)),
                             ".neff_cache")
    try:
        os.makedirs(cache_dir, exist_ok=True)
    except OSError:
        cache_dir = None
    _orig_compile = bu.compile_bir_kernel

    def cached_compile(bir_json, tmpdir, neff_name="file.neff"):
        if cache_dir is None:
            return _orig_compile(bir_json, tmpdir, neff_name)
        h = hashlib.sha256(bir_json).hexdigest()[:24]
        cpath = os.path.join(cache_dir, f"{h}.neff")
        out = os.path.join(tmpdir, neff_name)
        if os.path.exists(cpath):
            shutil.copyfile(cpath, out)
            return out
        r = _orig_compile(bir_json, tmpdir, neff_name)
        try:
            shutil.copyfile(r, cpath)
        except OSError:
            pass
        return r

    bu.compile_bir_kernel = cached_compile
    b2j.compile_bir_kernel = cached_compile


def _split_excess_waits(nc, max_waits=1):
    """walrus in this env allows only a couple of sync-wait slots per
    instruction; move excess waits onto preceding same-engine NOPs."""
    from concourse import mybir
    idx = 0
    for f in nc.m.functions:
        for bb in f.blocks:
            new = []
            changed = False
            for inst in bb.instructions:
                si = inst.sync_info
                waits = list(si.on_wait) if si is not None and si.on_wait else []
                if len(waits) > max_waits:
                    changed = True
                    for w in waits[:-max_waits]:
                        idx += 1
                        nop = mybir.InstNoOp(name=f"wsplit_{idx}", ins=[], outs=[])
                        nop.engine = inst.engine
                        nop.sync_info = mybir.SyncInfo(on_wait=[w], on_update=[])
                        new.append(nop)
                    si.on_wait = waits[-max_waits:]
                new.append(inst)
            if changed:
                bb.instructions = new


# ---------------------------------------------------------------------------
# device program
# ---------------------------------------------------------------------------

_NC_CACHE = {}


def _build_nc(split_waits=True):
    key = ("nc", split_waits)
    if key in _NC_CACHE:
        return _NC_CACHE[key]
    _apply_patches()
    import concourse.bass as bass
    import concourse.tile as tile
    from concourse import mybir
    from contextlib import ExitStack

    f32 = mybir.dt.float32
    bf16 = mybir.dt.bfloat16
    i16 = mybir.dt.int16
    u8 = mybir.dt.uint8
    fp8e4 = mybir.dt.float8e4
    DR = mybir.MatmulPerfMode.DoubleRow
    RELU = mybir.ActivationFunctionType.Relu
    EXP = mybir.ActivationFunctionType.Exp
    COPY = mybir.ActivationFunctionType.Copy
    LN = mybir.ActivationFunctionType.Ln
    ADD = mybir.AluOpType.add
    MULT = mybir.AluOpType.mult
    MAX = mybir.AluOpType.max

    nc = bass.Bass()

    xf_d = nc.declare_dram_parameter("xf", [C, HP * WP], bf16, isOutput=False)
    xl_d = nc.declare_dram_parameter("xl", [C, LOCP * WP], bf16, isOutput=False)
    # conv weights, 2-row-packed: taps (dr0|dr1) on 128 partitions, dr2 alone
    wpre_d = nc.declare_dram_parameter("wpre", [2 * C, 3 * C], bf16, isOutput=False)
    wpre2_d = nc.declare_dram_parameter("wpre2", [C, 3 * C], bf16, isOutput=False)
    b1_d = nc.declare_dram_parameter("b1", [C, 1], f32, isOutput=False)
    wfin_d = nc.declare_dram_parameter("wfin", [2 * C, 3 * C], bf16, isOutput=False)
    wfin2_d = nc.declare_dram_parameter("wfin2", [C, 3 * C], bf16, isOutput=False)
    b2_d = nc.declare_dram_parameter("b2", [C, 1], f32, isOutput=False)
    wq_d = nc.declare_dram_parameter("wq", [C + 1, CQK], bf16, isOutput=False)
    wk_d = nc.declare_dram_parameter("wk", [C + 1, CQK], bf16, isOutput=False)
    wv_d = nc.declare_dram_parameter("wv", [C + 1, C + 1], bf16, isOutput=False)
    ones_d = nc.declare_dram_parameter("ones1", [1, C], f32, isOutput=False)
    m2_d = nc.declare_dram_parameter("m2", [C, 2 * W], f32, isOutput=False)
    out_d = nc.declare_dram_parameter("out", [C, ROWS * W], f32, isOutput=True)

    with tile.TileContext(nc) as tc, ExitStack() as ctx:
        consts = ctx.enter_context(tc.tile_pool(name="consts", bufs=1))
        bigs = ctx.enter_context(tc.tile_pool(name="bigs", bufs=1))

        # --- constants ---
        wpre_sb = consts.tile([2 * C, 3 * C], bf16)
        wpre2_sb = consts.tile([C, 3 * C], bf16)
        wfin_sb = consts.tile([2 * C, 3 * C], bf16)
        wfin2_sb = consts.tile([C, 3 * C], bf16)
        b1_sb = consts.tile([C, 1], f32)
        b2_sb = consts.tile([C, 1], f32)
        wq_sb = consts.tile([C + 1, CQK], bf16)
        wk_sb = consts.tile([C + 1, CQK], bf16)
        wv_sb = consts.tile([C + 1, C + 1], bf16)
        ones_sb = consts.tile([1, C], f32)
        m2_sb = consts.tile([C, 2 * W], f32)
        # conv inputs first so the PE can start ASAP (DMA issue is serial)
        nc.sync.dma_start(out=wpre_sb, in_=wpre_d[:])
        nc.sync.dma_start(out=wpre2_sb, in_=wpre2_d[:])
        nc.sync.dma_start(out=b1_sb, in_=b1_d[:])

        # --- x frames, rows dr0 on partitions 0-63 / dr1 (shift 1 row) on 64-127
        xf_sb = bigs.tile([2 * C, HP * WP], bf16)
        xl_sb = bigs.tile([2 * C, LOCP * WP], bf16)
        nc.sync.dma_start(out=xl_sb[0:C, :], in_=xl_d[:])
        nc.sync.dma_start(out=xl_sb[C:2 * C, 0:(LOCP - 1) * WP],
                          in_=xl_d[:, WP:LOCP * WP])
        nc.sync.dma_start(out=wq_sb, in_=wq_d[:])
        nc.sync.dma_start(out=wk_sb, in_=wk_d[:])
        nc.sync.dma_start(out=wv_sb, in_=wv_d[:])
        for r0, r1 in [(0, 20), (20, 40), (40, 60), (60, 80), (80, HP)]:
            nc.sync.dma_start(out=xf_sb[0:C, r0 * WP:r1 * WP],
                              in_=xf_d[:, r0 * WP:r1 * WP])
            s0, s1 = min(r0 + 1, HP), min(r1 + 1, HP)
            nc.sync.dma_start(out=xf_sb[C:2 * C, (s0 - 1) * WP:(s1 - 1) * WP],
                              in_=xf_d[:, s0 * WP:s1 * WP])
        nc.sync.dma_start(out=wfin_sb, in_=wfin_d[:])
        nc.sync.dma_start(out=wfin2_sb, in_=wfin2_d[:])
        nc.sync.dma_start(out=b2_sb, in_=b2_d[:])
        nc.sync.dma_start(out=ones_sb, in_=ones_d[:])
        nc.sync.dma_start(out=m2_sb, in_=m2_d[:])

        xf3 = xf_sb.rearrange("p (r c) -> p r c", c=WP)
        xl3 = xl_sb.rearrange("p (r c) -> p r c", c=WP)

        ya_sb = bigs.tile([C + 1, N], bf16)       # y_aug (full sample)
        yla_sb = bigs.tile([C + 1, NLOC], bf16)   # y_aug (local window)
        ylf_sb = bigs.tile([C, NLOC], f32)        # y local fp32 (residual)
        k_sb = bigs.tile([112, N], bf16)          # k at offsets 0/32/64/96
        q_sb = bigs.tile([112, NLOC], bf16)
        vt_sb = bigs.tile([128, NJB * VBP], bf16)   # 80-wide slots, 65 used
        of_sb = bigs.tile([C, NLOC], f32)
        ofp_sb = bigs.tile([2 * C, LOCROWS * WP], bf16)  # 2-row-packed padded
        out_sb = bigs.tile([C, ROWS * W], f32)

        nc.gpsimd.memset(ya_sb[C:C + 1, :], 1.0)
        nc.gpsimd.memset(yla_sb[C:C + 1, :], 1.0)
        nc.gpsimd.memset(ofp_sb[:], 0.0)

        of3 = of_sb.rearrange("p (r c) -> p r c", c=W)
        m23 = m2_sb.rearrange("p (r c) -> p r c", c=W)
        ofp3 = ofp_sb.rearrange("p (r c) -> p r c", c=WP)

        def conv6(ps, x3, r, nr, wA, wB):
            """2-row-packed 3x3 conv: output rows r..r+nr of the padded frame."""
            for ds in range(3):
                nc.tensor.matmul(
                    ps[:, :nr * W],
                    wA[:, ds * C:(ds + 1) * C],
                    x3[:, r:r + nr, ds:ds + W],
                    start=(ds == 0), stop=False,
                )
            for ds in range(3):
                nc.tensor.matmul(
                    ps[:, :nr * W],
                    wB[:, ds * C:(ds + 1) * C],
                    x3[0:C, r + 2:r + 2 + nr, ds:ds + W],
                    start=False, stop=(ds == 2),
                )

        with tc.tile_pool(name="psA", bufs=3, space="PSUM") as psA, \
             tc.tile_pool(name="kq_ps", bufs=2, space="PSUM") as kq_ps, \
             tc.tile_pool(name="vt_ps", bufs=2, space="PSUM") as vt_ps:

            def k_chunk(kc):
                ps2 = kq_ps.tile([CQK, 512], f32, tag="kq")
                nc.tensor.matmul(ps2[:], wk_sb[:],
                                 ya_sb[:, kc * 512:(kc + 1) * 512],
                                 start=True, stop=True)
                nc.vector.tensor_copy(out=k_sb[0:CQK, kc * 512:(kc + 1) * 512],
                                      in_=ps2[:])

            def vt_group(g):
                ps3 = vt_ps.tile([128, 6 * VB], f32, tag="vt")
                for t in range(6):
                    jb = g * 6 + t
                    nc.tensor.matmul(
                        ps3[:, t * VB:(t + 1) * VB],
                        ya_sb[:, jb * JB:(jb + 1) * JB],
                        wv_sb[:], start=True, stop=True,
                    )
                vt_out = vt_sb.rearrange("p (b m) -> p b m", m=VBP)
                out_ap = vt_out[:, g * 6:(g + 1) * 6, 0:VB]
                if g % 2 == 0:
                    nc.scalar.activation(out=out_ap, in_=ps3[:], func=COPY)
                else:
                    nc.vector.tensor_copy(out=out_ap, in_=ps3[:])

            # --- P2: pre-conv over local window -> yla_sb, ylf_sb ---
            for m, nr in [(0, 4), (4, 4), (8, 4), (12, 4), (16, 4), (20, 4),
                          (24, 2)]:
                ps = psA.tile([C, 4 * W], f32, tag="conv_ps")
                conv6(ps, xl3, m, nr, wpre_sb, wpre2_sb)
                nc.scalar.activation(
                    out=yla_sb[0:C, m * W:(m + nr) * W],
                    in_=ps[:, :nr * W], func=RELU, bias=b1_sb[:, 0:1], scale=1.0,
                )
                nc.vector.tensor_scalar(
                    out=ylf_sb[:, m * W:(m + nr) * W], in0=ps[:, :nr * W],
                    scalar1=b1_sb[:, 0:1], scalar2=0.0, op0=ADD, op1=MAX,
                )

            # --- q projection ---
            ioff = 0
            for sz in NI_SIZES:
                ps = kq_ps.tile([CQK, 512], f32, tag="kq")
                nc.tensor.matmul(ps[:, :sz], wq_sb[:], yla_sb[:, ioff:ioff + sz],
                                 start=True, stop=True)
                nc.vector.tensor_copy(out=q_sb[0:CQK, ioff:ioff + sz],
                                      in_=ps[:, :sz])
                ioff += sz
            for t in range(1, 4):
                nc.sync.dma_start(out=q_sb[32 * t:32 * t + CQK, :],
                                  in_=q_sb[0:CQK, :])

            # --- P1 full-frame pre-conv interleaved with k / vT projections ---
            # conv chunk c covers ya cols [384c, 384c+384); k chunk kc needs
            # cols through 512(kc+1); vT group g needs cols through 768(g+1).
            kc_next = 0
            vt_next = 0
            for c4 in range(24):
                ps = psA.tile([C, 4 * W], f32, tag="conv_ps")
                conv6(ps, xf3, c4 * 4, 4, wpre_sb, wpre2_sb)
                nc.scalar.activation(
                    out=ya_sb[0:C, c4 * 4 * W:(c4 + 1) * 4 * W],
                    in_=ps[:], func=RELU, bias=b1_sb[:, 0:1], scale=1.0,
                )
                cols = 384 * c4  # cols complete once the PREVIOUS chunk's
                # evacuation is ordered; stay one chunk behind for overlap
                while (kc_next + 1) * 512 <= cols:
                    k_chunk(kc_next)
                    kc_next += 1
                while (vt_next + 1) * 768 <= cols:
                    vt_group(vt_next)
                    vt_next += 1
                if c4 == 12:
                    # k cols 0-4096 done; replicate the first half early
                    for t in range(1, 4):
                        nc.sync.dma_start(out=k_sb[32 * t:32 * t + CQK, 0:4096],
                                          in_=k_sb[0:CQK, 0:4096])
            for kc in range(kc_next, 18):
                k_chunk(kc)
            for g in range(vt_next, 12):
                vt_group(g)
            for t in range(1, 4):
                nc.sync.dma_start(out=k_sb[32 * t:32 * t + CQK, 4096:N],
                                  in_=k_sb[0:CQK, 4096:N])

        # --- P4: attention + interleaved final conv ---
        # PSUM: et single tiles [128,512] x6 (6 banks; bc/fc transients share
        # the same tag/slots) + acc x2 (2 banks) = 8.
        vt2 = vt_sb.rearrange("p (b m) -> p b m", m=VBP)
        with tc.tile_pool(name="et_ps", bufs=6, space="PSUM") as et_ps, \
             tc.tile_pool(name="acc_ps", bufs=2, space="PSUM") as acc_ps, \
             tc.tile_pool(name="p_pool", bufs=6) as p_pool, \
             tc.tile_pool(name="ep_pool", bufs=2) as ep_pool:

            def pv_pair(acc, u, p2, NI):
                for h in range(2):
                    jb = 2 * u + h
                    nc.tensor.matmul(acc[:, :NI], vt2[:, jb, 0:VB],
                                     p2[:, h * 512:h * 512 + NI],
                                     start=(jb == 0), stop=(jb == NJB - 1))

            def make_epilogue(ci, NI, ioff, acc, row_done):
                r = ep_pool.tile([1, 512], f32, tag="r")

                def epi_act():
                    # gamma/s as exp(-ln s) on the ACT engine (1/gamma is
                    # folded into wv's ones column; max rel err ~4e-5)
                    lns = ep_pool.tile([1, 512], f32, tag="lns")
                    nc.scalar.activation(out=lns[:, :NI], in_=acc[C:C + 1, :NI],
                                         func=LN)
                    nc.scalar.activation(out=r[:, :NI], in_=lns[:, :NI],
                                         func=EXP, scale=-1.0)

                def epi_mid():
                    # of = acc[0:64] * (gamma / s) + y_loc
                    bc = et_ps.tile([C, 512], f32, tag="et")
                    nc.tensor.matmul(bc[:, :NI], ones_sb[:], r[:, :NI],
                                     start=True, stop=True)
                    rb = ep_pool.tile([C, 512], f32, tag="rb")
                    nc.scalar.activation(out=rb[:, :NI], in_=bc[:, :NI],
                                         func=COPY)
                    nc.vector.tensor_tensor(out=of_sb[:, ioff:ioff + NI],
                                            in0=acc[0:C, :NI], in1=rb[:, :NI],
                                            op=MULT)
                    nc.gpsimd.tensor_tensor(out=of_sb[:, ioff:ioff + NI],
                                            in0=of_sb[:, ioff:ioff + NI],
                                            in1=ylf_sb[:, ioff:ioff + NI],
                                            op=ADD)
                    # mask out-of-image halo rows; stream completed rows into
                    # the 2-row-packed padded layout for the final conv
                    r1 = (ioff + NI) // W if ci < len(NI_SIZES) - 1 else LOCROWS
                    if ci == 0:
                        nc.gpsimd.tensor_tensor(
                            out=of3[:, 0:1, :], in0=of3[:, 0:1, :],
                            in1=m23[:, 0:1, :], op=MULT)
                    if ci == len(NI_SIZES) - 1:
                        nc.gpsimd.tensor_tensor(
                            out=of3[:, LOCROWS - 1:LOCROWS, :],
                            in0=of3[:, LOCROWS - 1:LOCROWS, :],
                            in1=m23[:, 1:2, :], op=MULT)
                    r0 = row_done
                    if r1 > r0:
                        nc.gpsimd.tensor_copy(out=ofp3[0:C, r0:r1, 1:1 + W],
                                              in_=of3[:, r0:r1, :])
                        s0 = max(r0, 1)
                        nc.gpsimd.tensor_copy(
                            out=ofp3[C:2 * C, s0 - 1:r1 - 1, 1:1 + W],
                            in_=of3[:, s0:r1, :])

                def epi_fc():
                    # final conv chunks whose input rows are now complete
                    for ch in FC_SCHED.get(ci, []):
                        ps = et_ps.tile([C, 512], f32, tag="et")
                        for ds in range(3):
                            nc.tensor.matmul(
                                ps[:, :4 * W],
                                wfin_sb[:, ds * C:(ds + 1) * C],
                                ofp3[:, ch * 4:ch * 4 + 4, ds:ds + W],
                                start=(ds == 0), stop=False,
                            )
                        for ds in range(3):
                            nc.tensor.matmul(
                                ps[:, :4 * W],
                                wfin2_sb[:, ds * C:(ds + 1) * C],
                                ofp3[0:C, ch * 4 + 2:ch * 4 + 6, ds:ds + W],
                                start=False, stop=(ds == 2),
                            )
                        nc.scalar.activation(
                            out=out_sb[:, ch * 4 * W:(ch + 1) * 4 * W],
                            in_=ps[:, :4 * W], func=RELU, bias=b2_sb[:, 0:1],
                            scale=1.0,
                        )
                        nc.sync.dma_start(
                            out=out_d[:, ch * 4 * W:(ch + 1) * 4 * W],
                            in_=out_sb[:, ch * 4 * W:(ch + 1) * 4 * W])
                return epi_act, epi_mid, epi_fc

            ioff = 0
            row_done = 0     # of rows fully written so far
            pending_epi = None
            u_glob = 0       # global pair index (exp engine round-robin)
            for ci, NI in enumerate(NI_SIZES):
                acc = acc_ps.tile([VB, 512], f32, tag="acc")
                pend = []    # (u, p2_tile) awaiting PV
                for g in range(NRND):
                    ets = []
                    for t in range(4):
                        jb = 4 * g + t
                        et = et_ps.tile([128, 512], f32, tag="et")
                        nc.tensor.matmul(
                            et[:, :NI],
                            k_sb[32 * t:32 * t + CQK, jb * JB:(jb + 1) * JB],
                            q_sb[32 * t:32 * t + CQK, ioff:ioff + NI],
                            start=True, stop=True,
                            tile_position=(32 * t, 0),
                        )
                        ets.append(et)
                    for t, et in enumerate(ets):
                        if t % 2 == 0:
                            p2 = p_pool.tile([128, 1024], bf16, tag="p")
                        p_ap = p2[:, (t % 2) * 512:(t % 2) * 512 + NI]
                        if u_glob % 2 == 0:
                            nc.scalar.activation(out=p_ap, in_=et[:, :NI],
                                                 func=EXP)
                        else:
                            nc.vector.tensor_scalar(
                                out=p_ap.bitcast(i16), in0=et[:, :NI],
                                scalar1=A16, scalar2=B16, op0=MULT, op1=ADD)
                        u_glob += 1
                        if t % 2 == 1:
                            pend.append((2 * g + t // 2, p2))
                    if g == 0 and pending_epi is not None:
                        pending_epi[0]()
                    if g == 4 and pending_epi is not None:
                        pending_epi[1]()
                    if g == 10 and pending_epi is not None:
                        pending_epi[2]()
                        pending_epi = None
                    if g > 0:
                        for u, p2 in pend[:2]:
                            pv_pair(acc, u, p2, NI)
                        pend = pend[2:]
                for u, p2 in pend:
                    pv_pair(acc, u, p2, NI)

                pending_epi = make_epilogue(ci, NI, ioff, acc, row_done)
                row_done = (ioff + NI) // W if ci < len(NI_SIZES) - 1 else LOCROWS
                ioff += NI
            pending_epi[0]()
            pending_epi[1]()
            pending_epi[2]()

    if split_waits:
        _split_excess_waits(nc)
    _NC_CACHE[key] = nc
    return nc


# ---------------------------------------------------------------------------
# host-side prep + launch
# ---------------------------------------------------------------------------

def _pack2row(wf):
    """[O,I,3,3] fused conv weight -> ([2C,3C] dr0|dr1 packed, [C,3C] dr2)."""
    wt = wf.transpose(1, 2, 3, 0)            # [cin, dr, ds, cout]
    wA = np.concatenate([wt[:, 0], wt[:, 1]], axis=0).reshape(2 * C, 3 * C)
    wB = wt[:, 2].reshape(C, 3 * C)
    return wA.astype(BF16), wB.astype(BF16)


def _prep_in_maps(x, w_pre, bn1_g, bn1_b, bn1_m, bn1_v, wq, bq, wk, bk, wv, bv,
                  w_fin, bn2_g, bn2_b, bn2_m, bn2_v, gamma):
    x = np.asarray(x, np.float32)
    inv1 = 1.0 / np.sqrt(np.asarray(bn1_v, np.float32) + EPS)
    s1 = np.asarray(bn1_g, np.float32) * inv1
    wpre_f = np.asarray(w_pre, np.float32) * s1[:, None, None, None]
    b1f = np.asarray(bn1_b, np.float32) - np.asarray(bn1_m, np.float32) * s1
    inv2 = 1.0 / np.sqrt(np.asarray(bn2_v, np.float32) + EPS)
    s2 = np.asarray(bn2_g, np.float32) * inv2
    wfin_f = np.asarray(w_fin, np.float32) * s2[:, None, None, None]
    b2f = np.asarray(bn2_b, np.float32) - np.asarray(bn2_m, np.float32) * s2

    wpreA, wpreB = _pack2row(wpre_f)
    wfinA, wfinB = _pack2row(wfin_f)

    gma = float(np.asarray(gamma, np.float32).reshape(-1)[0])
    wq2 = np.asarray(wq, np.float32).reshape(CQK, C)
    wk2 = np.asarray(wk, np.float32).reshape(CQK, C)
    wv2 = np.asarray(wv, np.float32).reshape(C, C)
    wq_aug = np.concatenate([wq2.T, np.asarray(bq, np.float32)[None, :]], 0).astype(BF16)
    wk_aug = np.concatenate([wk2.T, np.asarray(bk, np.float32)[None, :]], 0).astype(BF16)
    wv_aug = np.zeros((C + 1, C + 1), np.float32)
    if gma != 0.0:
        wv_aug[0:C, 0:C] = wv2.T
        wv_aug[C, 0:C] = np.asarray(bv, np.float32)
        wv_aug[C, C] = 1.0 / gma
    else:
        # gamma == 0: attention contributes nothing; make of = 0 + y by
        # zeroing v and keeping the denominator finite.
        wv_aug[C, C] = 1.0
    wv_aug = wv_aug.astype(BF16)

    ones1 = np.ones((1, C), np.float32)
    b1f = b1f.reshape(C, 1)
    b2f = b2f.reshape(C, 1)

    xpad = np.zeros((B, C, HP, WP), np.float32)
    xpad[:, :, 1:1 + H, 1:1 + W] = x
    xpad_bf = xpad.astype(BF16)

    in_maps = []
    for core in range(8):
        b, qc = divmod(core, QCH)
        xf = xpad_bf[b].reshape(C, HP * WP)
        # local window: image rows [24q-2, 24q+26) = padded rows [24q-1, 24q+27)
        xl = np.zeros((C, LOCP, WP), np.float32)
        pr0 = ROWS * qc - 1
        lo = max(0, -pr0)
        hi = min(LOCP, HP - pr0)
        xl[:, lo:hi, :] = xpad[b, :, pr0 + lo:pr0 + hi, :]
        xl = xl.reshape(C, LOCP * WP).astype(BF16)
        m2 = np.ones((C, 2 * W), np.float32)
        if qc == 0:
            m2[:, 0:W] = 0.0
        if qc == QCH - 1:
            m2[:, W:2 * W] = 0.0
        in_maps.append({
            "xf": xf, "xl": xl, "wpre": wpreA, "wpre2": wpreB, "b1": b1f,
            "wfin": wfinA, "wfin2": wfinB, "b2": b2f, "wq": wq_aug,
            "wk": wk_aug, "wv": wv_aug, "ones1": ones1, "m2": m2,
        })
    return in_maps


def kernel(**inputs):
    from concourse.bass_utils import run_bass_kernel_spmd

    nc = _build_nc()
    in_maps = _prep_in_maps(**inputs)
    res = run_bass_kernel_spmd(nc, in_maps, list(range(8)))
    out = np.zeros((B, C, H, W), np.float32)
    for core in range(8):
        b, qc = divmod(core, QCH)
        out[b, :, ROWS * qc:ROWS * (qc + 1), :] = \
            res.results[core]["out"].reshape(C, ROWS, W)
    return out


# revision 52
# speedup vs baseline: 1.0082x; 1.0082x over previous
"""Trainium2 Bass kernel for nn_AttentionLayer (pre-conv + self-attention + final conv).

Sharding: 8 cores = 2 samples x 4 query-row chunks. Each core computes the
full pre-conv y for its sample (k/v need all N=9216 positions), attention for
its 26-row query window (24 own rows + 1 halo row each side for the final
3x3 conv), and the final conv for its 24 output rows.

Perf structure (~253us vs the 405us baseline; rel err 4.1e-3):
- The PE clock is HAM-throttled to 1.2 GHz by default and un-throttles to
  2.4 GHz only under sustained dense attention-round activity; any multi-us
  stall re-throttles it. The attention loop never stalls the PE: energy
  rounds run one round ahead of PV, and the per-chunk epilogue is emitted in
  three pieces (recip at round 0, broadcast/residual at round 4, final-conv
  at round 10 of the NEXT i-chunk) so chunk transitions stay gap-free and
  the whole attention phase holds K=8/8 for a continuous ~130us.
- Energy matmuls use tile_position row groups (K=16 at offsets 0/32/64/96),
  each writing its own 1-bank PSUM tile; bc/fc transients share the same
  6-buffer PSUM tag so all 8 banks are used without blocking rotations.
- exp splits across the two PSUM-capable engines in strict alternation: ACT
  does true Exp, DVE a Schraudolph bit-trick exp (i16 = round(128*log2e*x +
  16256 - 5.5) bitcast to bf16; ~3% max elem err, diluted ~90x by the very
  diffuse softmax). Warm rounds are exp-bound at ~1.3us per 4 j-blocks.
- Softmax denominator comes from an augmented ones-column in wv (scaled by
  1/gamma); gamma/s is computed as exp(-ln s) on ACT (~4e-5 rel err) and
  broadcast across channels with a K=1 ones matmul on the PE instead of a
  DRAM round trip.
- Final 3x3 conv is 2-row-packed (6 matmuls instead of 9) from a
  double-height padded buffer filled by the Pool engine; its chunks are
  interleaved into the attention stream as their input rows complete.
- DMA emission order puts conv weights + the local window first so the PE
  starts ~12us in.

Measured dead ends: fp8 DoubleRow PV (accurate, but DoubleRow matmuls never
fire the HAM un-throttle -> whole attention at 1.2 GHz: 346-480us); pair-
width exp instructions over PSUM et pairs (serializes PE<->exp: 273us); 5/9
ACT/DVE exp split (266us); dummy/sparse energy matmuls in the conv phase to
pre-fire HAM (sparse ones don't trigger it, dense dummies hang the device);
early-attention overlap into the conv phase at psA bufs=2 (262us).
"""

import os
import hashlib
import shutil

import numpy as np
import ml_dtypes

BF16 = ml_dtypes.bfloat16
EPS = 1e-5

B, C, CQK, H, W = 2, 64, 16, 96, 96
N = H * W                       # 9216
QCH = 4                         # query chunks per sample
ROWS = H // QCH                 # 24 rows per core
LOCROWS = ROWS + 2              # 26 (with halo)
NLOC = LOCROWS * W              # 2496
HP, WP = H + 2, W + 2           # 98x98 padded frame
LOCP = LOCROWS + 2              # 28 padded local rows
NI_SIZES = [512, 512, 512, 512, 448]   # i-chunks over NLOC
JB = 128                        # j-block height
NJB = N // JB                   # 72
NPAIR = NJB // 2                # 36 DoubleRow j-block pairs
NRND = 18                       # rounds per i-chunk (2 pairs = 4 jb each)
VB = C + 1                      # 65
VBP = 80                        # padded vT slot width (DoubleRow step%16==0)

L2E = 1.4426950408889634
A16 = 128.0 * L2E
B16 = 127.0 * 128.0 - 5.5       # calibrated Schraudolph offset (bf16 variant)
A8 = 8.0 * L2E
B8 = 7.0 * 8.0 - 0.5            # calibrated fp8e4 Schraudolph offset

# final-conv chunks that become ready after attention i-chunk ci
FC_SCHED = {1: [0, 1], 2: [2], 3: [3], 4: [4, 5]}


# ---------------------------------------------------------------------------
# framework patches (self-contained)
# ---------------------------------------------------------------------------

def _apply_patches():
    import concourse.tile as tile
    import concourse.bass_utils as bu
    import concourse.bass2jax as b2j
    from concourse import mybir

    # 1) walrus in this env rejects >1-2 sync waits on the final Drain
    #    (CTRL_NO_STRUCT): split waits into single-wait nops.
    def _drain_and_barrier_split(self, tick_clock, wait_clock):
        nc = self.nc
        probe = nc.sync.nop()
        wait_clock.add_sem_waits(
            probe.ins, tile.ScopedClock({None: tick_clock.global_clock})
        )
        waits = list(probe.ins.sync_info.on_wait) if probe.ins.sync_info else []
        if probe.ins.sync_info is not None:
            probe.ins.sync_info.on_wait = []
        for w in waits[:-1]:
            nop = nc.sync.nop()
            if nop.ins.sync_info is None:
                nop.ins.sync_info = mybir.SyncInfo(on_wait=[w], on_update=[])
            else:
                nop.ins.sync_info.on_wait.append(w)
        drain_inst = nc.sync.drain()
        if waits:
            if drain_inst.ins.sync_info is None:
                drain_inst.ins.sync_info = mybir.SyncInfo(
                    on_wait=[waits[-1]], on_update=[]
                )
            else:
                drain_inst.ins.sync_info.on_wait.append(waits[-1])
        nc.all_engine_barrier()
        assert self.sems is not None
        popped = nc._tile_sem_poison_stack.pop()
        assert popped is self._sem_poison
        nc.clear_and_free_semaphores(list(self.sems.allocated().values()))
        nc.all_engine_barrier()

    tile.TileContext._drain_and_barrier = _drain_and_barrier_split

    # 2) NEFF disk cache keyed by BIR hash (compile is deterministic).
    cache_dir = os.path.join(os.path.dirname(os.path.abspath(__file__)),
                             ".neff_cache")
    try:
        os.makedirs(cache_dir, exist_ok=True)
    except OSError:
        cache_dir = None
    _orig_compile = bu.compile_bir_kernel

    def cached_compile(bir_json, tmpdir, neff_name="file.neff"):
        if cache_dir is None:
            return _orig_compile(bir_json, tmpdir, neff_name)
        h = hashlib.sha256(bir_json).hexdigest()[:24]
        cpath = os.path.join(cache_dir, f"{h}.neff")
        out = os.path.join(tmpdir, neff_name)
        if os.path.exists(cpath):
            shutil.copyfile(cpath, out)
            return out
        r = _orig_compile(bir_json, tmpdir, neff_name)
        try:
            shutil.copyfile(r, cpath)
        except OSError:
            pass
        return r

    bu.compile_bir_kernel = cached_compile
    b2j.compile_bir_kernel = cached_compile


def _split_excess_waits(nc, max_waits=1):
    """walrus in this env allows only a couple of sync-wait slots per
    instruction; move excess waits onto preceding same-engine NOPs."""
    from concourse import mybir
    idx = 0
    for f in nc.m.functions:
        for bb in f.blocks:
            new = []
            changed = False
            for inst in bb.instructions:
                si = inst.sync_info
                waits = list(si.on_wait) if si is not None and si.on_wait else []
                if len(waits) > max_waits:
                    changed = True
                    for w in waits[:-max_waits]:
                        idx += 1
                        nop = mybir.InstNoOp(name=f"wsplit_{idx}", ins=[], outs=[])
                        nop.engine = inst.engine
                        nop.sync_info = mybir.SyncInfo(on_wait=[w], on_update=[])
                        new.append(nop)
                    si.on_wait = waits[-max_waits:]
                new.append(inst)
            if changed:
                bb.instructions = new


# ---------------------------------------------------------------------------
# device program
# ---------------------------------------------------------------------------

_NC_CACHE = {}


def _build_nc(split_waits=True):
    key = ("nc", split_waits)
    if key in _NC_CACHE:
        return _NC_CACHE[key]
    _apply_patches()
    import concourse.bass as bass
    import concourse.tile as tile
    from concourse import mybir
    from contextlib import ExitStack

    f32 = mybir.dt.float32
    bf16 = mybir.dt.bfloat16
    i16 = mybir.dt.int16
    u8 = mybir.dt.uint8
    fp8e4 = mybir.dt.float8e4
    DR = mybir.MatmulPerfMode.DoubleRow
    RELU = mybir.ActivationFunctionType.Relu
    EXP = mybir.ActivationFunctionType.Exp
    COPY = mybir.ActivationFunctionType.Copy
    LN = mybir.ActivationFunctionType.Ln
    ADD = mybir.AluOpType.add
    MULT = mybir.AluOpType.mult
    MAX = mybir.AluOpType.max

    nc = bass.Bass()

    xf_d = nc.declare_dram_parameter("xf", [C, HP * WP], bf16, isOutput=False)
    xl_d = nc.declare_dram_parameter("xl", [C, LOCP * WP], bf16, isOutput=False)
    # conv weights, 2-row-packed: taps (dr0|dr1) on 128 partitions, dr2 alone
    wpre_d = nc.declare_dram_parameter("wpre", [2 * C, 3 * C], bf16, isOutput=False)
    wpre2_d = nc.declare_dram_parameter("wpre2", [C, 3 * C], bf16, isOutput=False)
    b1_d = nc.declare_dram_parameter("b1", [C, 1], f32, isOutput=False)
    wfin_d = nc.declare_dram_parameter("wfin", [2 * C, 3 * C], bf16, isOutput=False)
    wfin2_d = nc.declare_dram_parameter("wfin2", [C, 3 * C], bf16, isOutput=False)
    b2_d = nc.declare_dram_parameter("b2", [C, 1], f32, isOutput=False)
    wq_d = nc.declare_dram_parameter("wq", [C + 1, CQK], bf16, isOutput=False)
    wk_d = nc.declare_dram_parameter("wk", [C + 1, CQK], bf16, isOutput=False)
    wv_d = nc.declare_dram_parameter("wv", [C + 1, C + 1], bf16, isOutput=False)
    ones_d = nc.declare_dram_parameter("ones1", [1, C], f32, isOutput=False)
    m2_d = nc.declare_dram_parameter("m2", [C, 2 * W], f32, isOutput=False)
    out_d = nc.declare_dram_parameter("out", [C, ROWS * W], f32, isOutput=True)

    with tile.TileContext(nc) as tc, ExitStack() as ctx:
        consts = ctx.enter_context(tc.tile_pool(name="consts", bufs=1))
        bigs = ctx.enter_context(tc.tile_pool(name="bigs", bufs=1))

        # --- constants ---
        wpre_sb = consts.tile([2 * C, 3 * C], bf16)
        wpre2_sb = consts.tile([C, 3 * C], bf16)
        wfin_sb = consts.tile([2 * C, 3 * C], bf16)
        wfin2_sb = consts.tile([C, 3 * C], bf16)
        b1_sb = consts.tile([C, 1], f32)
        b2_sb = consts.tile([C, 1], f32)
        wq_sb = consts.tile([C + 1, CQK], bf16)
        wk_sb = consts.tile([C + 1, CQK], bf16)
        wv_sb = consts.tile([C + 1, C + 1], bf16)
        ones_sb = consts.tile([1, C], f32)
        m2_sb = consts.tile([C, 2 * W], f32)
        # conv inputs first so the PE can start ASAP (DMA issue is serial)
        nc.sync.dma_start(out=wpre_sb, in_=wpre_d[:])
        nc.sync.dma_start(out=wpre2_sb, in_=wpre2_d[:])
        nc.sync.dma_start(out=b1_sb, in_=b1_d[:])

        # --- x frames, rows dr0 on partitions 0-63 / dr1 (shift 1 row) on 64-127
        xf_sb = bigs.tile([2 * C, HP * WP], bf16)
        xl_sb = bigs.tile([2 * C, LOCP * WP], bf16)
        nc.sync.dma_start(out=xl_sb[0:C, :], in_=xl_d[:])
        nc.sync.dma_start(out=xl_sb[C:2 * C, 0:(LOCP - 1) * WP],
                          in_=xl_d[:, WP:LOCP * WP])
        nc.sync.dma_start(out=wq_sb, in_=wq_d[:])
        nc.sync.dma_start(out=wk_sb, in_=wk_d[:])
        nc.sync.dma_start(out=wv_sb, in_=wv_d[:])
        for r0, r1 in [(0, 20), (20, 40), (40, 60), (60, 80), (80, HP)]:
            nc.sync.dma_start(out=xf_sb[0:C, r0 * WP:r1 * WP],
                              in_=xf_d[:, r0 * WP:r1 * WP])
            s0, s1 = min(r0 + 1, HP), min(r1 + 1, HP)
            nc.sync.dma_start(out=xf_sb[C:2 * C, (s0 - 1) * WP:(s1 - 1) * WP],
                              in_=xf_d[:, s0 * WP:s1 * WP])
        nc.sync.dma_start(out=wfin_sb, in_=wfin_d[:])
        nc.sync.dma_start(out=wfin2_sb, in_=wfin2_d[:])
        nc.sync.dma_start(out=b2_sb, in_=b2_d[:])
        nc.sync.dma_start(out=ones_sb, in_=ones_d[:])
        nc.sync.dma_start(out=m2_sb, in_=m2_d[:])

        xf3 = xf_sb.rearrange("p (r c) -> p r c", c=WP)
        xl3 = xl_sb.rearrange("p (r c) -> p r c", c=WP)

        ya_sb = bigs.tile([C + 1, N], bf16)       # y_aug (full sample)
        yla_sb = bigs.tile([C + 1, NLOC], bf16)   # y_aug (local window)
        ylf_sb = bigs.tile([C, NLOC], f32)        # y local fp32 (residual)
        k_sb = bigs.tile([112, N], bf16)          # k at offsets 0/32/64/96
        q_sb = bigs.tile([112, NLOC], bf16)
        vt_sb = bigs.tile([128, NJB * VBP], bf16)   # 80-wide slots, 65 used
        of_sb = bigs.tile([C, NLOC], f32)
        ofp_sb = bigs.tile([2 * C, LOCROWS * WP], bf16)  # 2-row-packed padded
        out_sb = bigs.tile([C, ROWS * W], f32)

        nc.gpsimd.memset(ya_sb[C:C + 1, :], 1.0)
        nc.gpsimd.memset(yla_sb[C:C + 1, :], 1.0)
        nc.gpsimd.memset(ofp_sb[:], 0.0)

        of3 = of_sb.rearrange("p (r c) -> p r c", c=W)
        m23 = m2_sb.rearrange("p (r c) -> p r c", c=W)
        ofp3 = ofp_sb.rearrange("p (r c) -> p r c", c=WP)

        def conv6(ps, x3, r, nr, wA, wB):
            """2-row-packed 3x3 conv: output rows r..r+nr of the padded frame."""
            for ds in range(3):
                nc.tensor.matmul(
                    ps[:, :nr * W],
                    wA[:, ds * C:(ds + 1) * C],
                    x3[:, r:r + nr, ds:ds + W],
                    start=(ds == 0), stop=False,
                )
            for ds in range(3):
                nc.tensor.matmul(
                    ps[:, :nr * W],
                    wB[:, ds * C:(ds + 1) * C],
                    x3[0:C, r + 2:r + 2 + nr, ds:ds + W],
                    start=False, stop=(ds == 2),
                )

        with tc.tile_pool(name="psA", bufs=3, space="PSUM") as psA, \
             tc.tile_pool(name="kq_ps", bufs=2, space="PSUM") as kq_ps, \
             tc.tile_pool(name="vt_ps", bufs=2, space="PSUM") as vt_ps:

            def k_chunk(kc):
                ps2 = kq_ps.tile([CQK, 512], f32, tag="kq")
                nc.tensor.matmul(ps2[:], wk_sb[:],
                                 ya_sb[:, kc * 512:(kc + 1) * 512],
                                 start=True, stop=True)
                nc.vector.tensor_copy(out=k_sb[0:CQK, kc * 512:(kc + 1) * 512],
                                      in_=ps2[:])

            def vt_group(g):
                ps3 = vt_ps.tile([128, 6 * VB], f32, tag="vt")
                for t in range(6):
                    jb = g * 6 + t
                    nc.tensor.matmul(
                        ps3[:, t * VB:(t + 1) * VB],
                        ya_sb[:, jb * JB:(jb + 1) * JB],
                        wv_sb[:], start=True, stop=True,
                    )
                vt_out = vt_sb.rearrange("p (b m) -> p b m", m=VBP)
                out_ap = vt_out[:, g * 6:(g + 1) * 6, 0:VB]
                if g % 2 == 0:
                    nc.scalar.activation(out=out_ap, in_=ps3[:], func=COPY)
                else:
                    nc.vector.tensor_copy(out=out_ap, in_=ps3[:])

            # --- P2: pre-conv over local window -> yla_sb, ylf_sb ---
            for m, nr in [(0, 4), (4, 4), (8, 4), (12, 4), (16, 4), (20, 4),
                          (24, 2)]:
                ps = psA.tile([C, 4 * W], f32, tag="conv_ps")
                conv6(ps, xl3, m, nr, wpre_sb, wpre2_sb)
                nc.scalar.activation(
                    out=yla_sb[0:C, m * W:(m + nr) * W],
                    in_=ps[:, :nr * W], func=RELU, bias=b1_sb[:, 0:1], scale=1.0,
                )
                nc.vector.tensor_scalar(
                    out=ylf_sb[:, m * W:(m + nr) * W], in0=ps[:, :nr * W],
                    scalar1=b1_sb[:, 0:1], scalar2=0.0, op0=ADD, op1=MAX,
                )

            # --- q projection ---
            ioff = 0
            for sz in NI_SIZES:
                ps = kq_ps.tile([CQK, 512], f32, tag="kq")
                nc.tensor.matmul(ps[:, :sz], wq_sb[:], yla_sb[:, ioff:ioff + sz],
                                 start=True, stop=True)
                nc.vector.tensor_copy(out=q_sb[0:CQK, ioff:ioff + sz],
                                      in_=ps[:, :sz])
                ioff += sz
            for t in range(1, 4):
                nc.sync.dma_start(out=q_sb[32 * t:32 * t + CQK, :],
                                  in_=q_sb[0:CQK, :])

            # --- P1 full-frame pre-conv interleaved with k / vT projections ---
            # conv chunk c covers ya cols [384c, 384c+384); k chunk kc needs
            # cols through 512(kc+1); vT group g needs cols through 768(g+1).
            kc_next = 0
            vt_next = 0
            for c4 in range(24):
                ps = psA.tile([C, 4 * W], f32, tag="conv_ps")
                conv6(ps, xf3, c4 * 4, 4, wpre_sb, wpre2_sb)
                nc.scalar.activation(
                    out=ya_sb[0:C, c4 * 4 * W:(c4 + 1) * 4 * W],
                    in_=ps[:], func=RELU, bias=b1_sb[:, 0:1], scale=1.0,
                )
                cols = 384 * c4  # cols complete once the PREVIOUS chunk's
                # evacuation is ordered; stay one chunk behind for overlap
                while (kc_next + 1) * 512 <= cols:
                    k_chunk(kc_next)
                    kc_next += 1
                while (vt_next + 1) * 768 <= cols:
                    vt_group(vt_next)
                    vt_next += 1
                if c4 == 12:
                    # k cols 0-4096 done; replicate the first half early
                    for t in range(1, 4):
                        nc.sync.dma_start(out=k_sb[32 * t:32 * t + CQK, 0:4096],
                                          in_=k_sb[0:CQK, 0:4096])
            for kc in range(kc_next, 18):
                k_chunk(kc)
            for g in range(vt_next, 12):
                vt_group(g)
            for t in range(1, 4):
                nc.sync.dma_start(out=k_sb[32 * t:32 * t + CQK, 4096:N],
                                  in_=k_sb[0:CQK, 4096:N])

        # --- P4: attention + interleaved final conv ---
        # PSUM: et single tiles [128,512] x6 (6 banks; bc/fc transients share
        # the same tag/slots) + acc x2 (2 banks) = 8.
        vt2 = vt_sb.rearrange("p (b m) -> p b m", m=VBP)
        with tc.tile_pool(name="et_ps", bufs=6, space="PSUM") as et_ps, \
             tc.tile_pool(name="acc_ps", bufs=2, space="PSUM") as acc_ps, \
             tc.tile_pool(name="p_pool", bufs=4) as p_pool, \
             tc.tile_pool(name="ep_pool", bufs=2) as ep_pool:

            def pv_pair(acc, u, p2, NI):
                for h in range(2):
                    jb = 2 * u + h
                    nc.tensor.matmul(acc[:, :NI], vt2[:, jb, 0:VB],
                                     p2[:, h * 512:h * 512 + NI],
                                     start=(jb == 0), stop=(jb == NJB - 1))

            def make_epilogue(ci, NI, ioff, acc, row_done):
                r = ep_pool.tile([1, 512], f32, tag="r")

                def epi_act():
                    # gamma/s as exp(-ln s) on the ACT engine (1/gamma is
                    # folded into wv's ones column; max rel err ~4e-5)
                    lns = ep_pool.tile([1, 512], f32, tag="lns")
                    nc.scalar.activation(out=lns[:, :NI], in_=acc[C:C + 1, :NI],
                                         func=LN)
                    nc.scalar.activation(out=r[:, :NI], in_=lns[:, :NI],
                                         func=EXP, scale=-1.0)

                def epi_mid():
                    # of = acc[0:64] * (gamma / s) + y_loc
                    bc = et_ps.tile([C, 512], f32, tag="et")
                    nc.tensor.matmul(bc[:, :NI], ones_sb[:], r[:, :NI],
                                     start=True, stop=True)
                    rb = ep_pool.tile([C, 512], f32, tag="rb")
                    nc.scalar.activation(out=rb[:, :NI], in_=bc[:, :NI],
                                         func=COPY)
                    nc.vector.tensor_tensor(out=of_sb[:, ioff:ioff + NI],
                                            in0=acc[0:C, :NI], in1=rb[:, :NI],
                                            op=MULT)
                    nc.gpsimd.tensor_tensor(out=of_sb[:, ioff:ioff + NI],
                                            in0=of_sb[:, ioff:ioff + NI],
                                            in1=ylf_sb[:, ioff:ioff + NI],
                                            op=ADD)
                    # mask out-of-image halo rows; stream completed rows into
                    # the 2-row-packed padded layout for the final conv
                    r1 = (ioff + NI) // W if ci < len(NI_SIZES) - 1 else LOCROWS
                    if ci == 0:
                        nc.gpsimd.tensor_tensor(
                            out=of3[:, 0:1, :], in0=of3[:, 0:1, :],
                            in1=m23[:, 0:1, :], op=MULT)
                    if ci == len(NI_SIZES) - 1:
                        nc.gpsimd.tensor_tensor(
                            out=of3[:, LOCROWS - 1:LOCROWS, :],
                            in0=of3[:, LOCROWS - 1:LOCROWS, :],
                            in1=m23[:, 1:2, :], op=MULT)
                    r0 = row_done
                    if r1 > r0:
                        nc.gpsimd.tensor_copy(out=ofp3[0:C, r0:r1, 1:1 + W],
                                              in_=of3[:, r0:r1, :])
                        s0 = max(r0, 1)
                        nc.gpsimd.tensor_copy(
                            out=ofp3[C:2 * C, s0 - 1:r1 - 1, 1:1 + W],
                            in_=of3[:, s0:r1, :])

                def epi_fc():
                    # final conv chunks whose input rows are now complete
                    for ch in FC_SCHED.get(ci, []):
                        ps = et_ps.tile([C, 512], f32, tag="et")
                        for ds in range(3):
                            nc.tensor.matmul(
                                ps[:, :4 * W],
                                wfin_sb[:, ds * C:(ds + 1) * C],
                                ofp3[:, ch * 4:ch * 4 + 4, ds:ds + W],
                                start=(ds == 0), stop=False,
                            )
                        for ds in range(3):
                            nc.tensor.matmul(
                                ps[:, :4 * W],
                                wfin2_sb[:, ds * C:(ds + 1) * C],
                                ofp3[0:C, ch * 4 + 2:ch * 4 + 6, ds:ds + W],
                                start=False, stop=(ds == 2),
                            )
                        nc.scalar.activation(
                            out=out_sb[:, ch * 4 * W:(ch + 1) * 4 * W],
                            in_=ps[:, :4 * W], func=RELU, bias=b2_sb[:, 0:1],
                            scale=1.0,
                        )
                        nc.sync.dma_start(
                            out=out_d[:, ch * 4 * W:(ch + 1) * 4 * W],
                            in_=out_sb[:, ch * 4 * W:(ch + 1) * 4 * W])
                return epi_act, epi_mid, epi_fc

            ioff = 0
            row_done = 0     # of rows fully written so far
            pending_epi = None
            u_glob = 0       # global pair index (exp engine round-robin)
            for ci, NI in enumerate(NI_SIZES):
                acc = acc_ps.tile([VB, 512], f32, tag="acc")
                pend = []    # (u, p2_tile) awaiting PV
                for g in range(NRND):
                    ets = []
                    for t in range(4):
                        jb = 4 * g + t
                        et = et_ps.tile([128, 512], f32, tag="et")
                        nc.tensor.matmul(
                            et[:, :NI],
                            k_sb[32 * t:32 * t + CQK, jb * JB:(jb + 1) * JB],
                            q_sb[32 * t:32 * t + CQK, ioff:ioff + NI],
                            start=True, stop=True,
                            tile_position=(32 * t, 0),
                        )
                        ets.append(et)
                    for t, et in enumerate(ets):
                        if t % 2 == 0:
                            p2 = p_pool.tile([128, 1024], bf16, tag="p")
                        p_ap = p2[:, (t % 2) * 512:(t % 2) * 512 + NI]
                        if u_glob % 2 == 0:
                            nc.scalar.activation(out=p_ap, in_=et[:, :NI],
                                                 func=EXP)
                        else:
                            nc.vector.tensor_scalar(
                                out=p_ap.bitcast(i16), in0=et[:, :NI],
                                scalar1=A16, scalar2=B16, op0=MULT, op1=ADD)
                        u_glob += 1
                        if t % 2 == 1:
                            pend.append((2 * g + t // 2, p2))
                    if g == 0 and pending_epi is not None:
                        pending_epi[0]()
                    if g == 4 and pending_epi is not None:
                        pending_epi[1]()
                    if g == 10 and pending_epi is not None:
                        pending_epi[2]()
                        pending_epi = None
                    if g > 0:
                        for u, p2 in pend[:2]:
                            pv_pair(acc, u, p2, NI)
                        pend = pend[2:]
                for u, p2 in pend:
                    pv_pair(acc, u, p2, NI)

                pending_epi = make_epilogue(ci, NI, ioff, acc, row_done)
                row_done = (ioff + NI) // W if ci < len(NI_SIZES) - 1 else LOCROWS
                ioff += NI
            pending_epi[0]()
            pending_epi[1]()
            pending_epi[2]()

    if split_waits:
        _split_excess_waits(nc)
    _NC_CACHE[key] = nc
    return nc


# ---------------------------------------------------------------------------
# host-side prep + launch
# ---------------------------------------------------------------------------

def _pack2row(wf):
    """[O,I,3,3] fused conv weight -> ([2C,3C] dr0|dr1 packed, [C,3C] dr2)."""
    wt = wf.transpose(1, 2, 3, 0)            # [cin, dr, ds, cout]
    wA = np.concatenate([wt[:, 0], wt[:, 1]], axis=0).reshape(2 * C, 3 * C)
    wB = wt[:, 2].reshape(C, 3 * C)
    return wA.astype(BF16), wB.astype(BF16)


def _prep_in_maps(x, w_pre, bn1_g, bn1_b, bn1_m, bn1_v, wq, bq, wk, bk, wv, bv,
                  w_fin, bn2_g, bn2_b, bn2_m, bn2_v, gamma):
    x = np.asarray(x, np.float32)
    inv1 = 1.0 / np.sqrt(np.asarray(bn1_v, np.float32) + EPS)
    s1 = np.asarray(bn1_g, np.float32) * inv1
    wpre_f = np.asarray(w_pre, np.float32) * s1[:, None, None, None]
    b1f = np.asarray(bn1_b, np.float32) - np.asarray(bn1_m, np.float32) * s1
    inv2 = 1.0 / np.sqrt(np.asarray(bn2_v, np.float32) + EPS)
    s2 = np.asarray(bn2_g, np.float32) * inv2
    wfin_f = np.asarray(w_fin, np.float32) * s2[:, None, None, None]
    b2f = np.asarray(bn2_b, np.float32) - np.asarray(bn2_m, np.float32) * s2

    wpreA, wpreB = _pack2row(wpre_f)
    wfinA, wfinB = _pack2row(wfin_f)

    gma = float(np.asarray(gamma, np.float32).reshape(-1)[0])
    wq2 = np.asarray(wq, np.float32).reshape(CQK, C)
    wk2 = np.asarray(wk, np.float32).reshape(CQK, C)
    wv2 = np.asarray(wv, np.float32).reshape(C, C)
    wq_aug = np.concatenate([wq2.T, np.asarray(bq, np.float32)[None, :]], 0).astype(BF16)
    wk_aug = np.concatenate([wk2.T, np.asarray(bk, np.float32)[None, :]], 0).astype(BF16)
    wv_aug = np.zeros((C + 1, C + 1), np.float32)
    if gma != 0.0:
        wv_aug[0:C, 0:C] = wv2.T
        wv_aug[C, 0:C] = np.asarray(bv, np.float32)
        wv_aug[C, C] = 1.0 / gma
    else:
        # gamma == 0: attention contributes nothing; make of = 0 + y by
        # zeroing v and keeping the denominator finite.
        wv_aug[C, C] = 1.0
    wv_aug = wv_aug.astype(BF16)

    ones1 = np.ones((1, C), np.float32)
    b1f = b1f.reshape(C, 1)
    b2f = b2f.reshape(C, 1)

    xpad = np.zeros((B, C, HP, WP), np.float32)
    xpad[:, :, 1:1 + H, 1:1 + W] = x
    xpad_bf = xpad.astype(BF16)

    in_maps = []
    for core in range(8):
        b, qc = divmod(core, QCH)
        xf = xpad_bf[b].reshape(C, HP * WP)
        # local window: image rows [24q-2, 24q+26) = padded rows [24q-1, 24q+27)
        xl = np.zeros((C, LOCP, WP), np.float32)
        pr0 = ROWS * qc - 1
        lo = max(0, -pr0)
        hi = min(LOCP, HP - pr0)
        xl[:, lo:hi, :] = xpad[b, :, pr0 + lo:pr0 + hi, :]
        xl = xl.reshape(C, LOCP * WP).astype(BF16)
        m2 = np.ones((C, 2 * W), np.float32)
        if qc == 0:
            m2[:, 0:W] = 0.0
        if qc == QCH - 1:
            m2[:, W:2 * W] = 0.0
        in_maps.append({
            "xf": xf, "xl": xl, "wpre": wpreA, "wpre2": wpreB, "b1": b1f,
            "wfin": wfinA, "wfin2": wfinB, "b2": b2f, "wq": wq_aug,
            "wk": wk_aug, "wv": wv_aug, "ones1": ones1, "m2": m2,
        })
    return in_maps


def kernel(**inputs):
    from concourse.bass_utils import run_bass_kernel_spmd

    nc = _build_nc()
    in_maps = _prep_in_maps(**inputs)
    res = run_bass_kernel_spmd(nc, in_maps, list(range(8)))
    out = np.zeros((B, C, H, W), np.float32)
    for core in range(8):
        b, qc = divmod(core, QCH)
        out[b, :, ROWS * qc:ROWS * (qc + 1), :] = \
            res.results[core]["out"].reshape(C, ROWS, W)
    return out
